# revision 18
# baseline (speedup 1.0000x reference)
"""Edge-parallel GNN discriminator kernel for 8 TRN2 NeuronCores.

Computes Y[e] = sigmoid(w * dot(Z[src[e]], Z[dst[e]]) + b) for E edges.

Strategy (edge-parallel, per the sharding hint):
  - Each of the 8 cores holds a full Z replica in HBM and processes E/8 edges.
  - Row gather uses the Anthropic dma_gather extended instruction (one 512B
    descriptor per row, descriptors spread over all 16 SDMA engines, Q7-pair
    descriptor generation). dma_gather takes int16 indices (< 32768), but
    N = 50000 rows: view Z as 25000 row-PAIRS (stride 1024B) and gather with
    q = row >> 1 from a base offset selected by row parity. Edges are sorted
    host-side into 4 segments by (src parity, dst parity) so each tile's two
    gathers use compile-time base offsets; the host inverse-permutes outputs.
  - dma_gather writes slot s -> partition s%128, block s//128. Per tile:
    zs, zd [128, T/128, 128] f32; DVE multiplies and 3D-reduces to
    dots [128, T/128]; one ACT sigmoid (scale=w, bias=b) over the accumulated
    result; contiguous DMA out.
  - All cores run one SPMD NEFF: segment capacities are the max over cores,
    padded to tile granularity with dummy slots (index 0) that are dropped
    host-side.
"""

import numpy as np

import concourse.bacc as bacc
import concourse.mybir as mybir
from concourse.tile import TileContext
from concourse.bass_utils import run_bass_kernel_spmd

N_CORES = 8
P = 128
D = 128
BIG_T = 896
SMALL_T = 128
DMA_SCRATCH = 16384
NUM_QUEUES = 4
SINGLE_PACKET = True
ACT_BLOCKS = 0
QUEUE_FROM_LANE = None


def _plan_tiles(n):
    """Tile sizes (each a multiple of SMALL_T) covering >= n slots."""
    n = max(n, SMALL_T)
    ts = [BIG_T] * (n // BIG_T)
    rem = n - (n // BIG_T) * BIG_T
    if rem:
        ts.append(((rem + SMALL_T - 1) // SMALL_T) * SMALL_T)
    return ts


def _build(nc, n_nodes, tiles, slots, gather_bufs=8, schedule=None, compute=True):
    """tiles: list of (src_parity, dst_parity, T) in slot order.

    schedule: optional explicit list of (ps, pd, T, slot_base); defaults to
    the running-offset schedule implied by `tiles` (used by perf rigs to
    repeat the tile schedule)."""
    ncols = slots // 16
    rcols = slots // 128
    Z = nc.dram_tensor("Z", [n_nodes, D], mybir.dt.float32, kind="ExternalInput")
    ISRC = nc.dram_tensor("isrc", [16, ncols], mybir.dt.int16, kind="ExternalInput")
    IDST = nc.dram_tensor("idst", [16, ncols], mybir.dt.int16, kind="ExternalInput")
    WB = nc.dram_tensor("wb", [P, 2], mybir.dt.float32, kind="ExternalInput")
    Y = nc.dram_tensor("y", [P, rcols], mybir.dt.float32, kind="ExternalOutput")

    # Z as row-pairs: [25000, 256]; parity r selects a 128-elem base offset.
    zview = Z[:].rearrange("(q r) d -> q (r d)", r=2)

    with TileContext(nc) as tc:
        with (
            tc.tile_pool(name="idxp", bufs=1) as idx_pool,
            tc.tile_pool(name="gath", bufs=gather_bufs) as gpool,
            tc.tile_pool(name="misc", bufs=1) as mpool,
        ):
            isrc_sb = idx_pool.tile([P, ncols], mybir.dt.int16, tag="isrc")
            idst_sb = idx_pool.tile([P, ncols], mybir.dt.int16, tag="idst")
            # Each Q7 core reads idxs from its own 16-partition window:
            # replicate the wrapped idx block into all 8 groups.
            for grp in range(8):
                nc.sync.dma_start(
                    out=isrc_sb[grp * 16:(grp + 1) * 16, :], in_=ISRC[:]
                )
                nc.sync.dma_start(
                    out=idst_sb[grp * 16:(grp + 1) * 16, :], in_=IDST[:]
                )

            wb_sb = mpool.tile([P, 2], mybir.dt.float32, tag="wb")
            nc.sync.dma_start(out=wb_sb[:], in_=WB[:])
            wb_dve = mpool.tile([P, 2], mybir.dt.float32, tag="wbd")
            nc.vector.tensor_copy(out=wb_dve[:], in_=wb_sb[:])

            res = mpool.tile([P, rcols], mybir.dt.float32, tag="res")
            act_scratch = mpool.tile([P, D], mybir.dt.float32, tag="actscr")
            if not compute:
                nc.vector.memset(res[:], 0)

            if schedule is None:
                schedule = []
                slot_base = 0
                for (ps, pd, T) in tiles:
                    schedule.append((ps, pd, T, slot_base))
                    slot_base += T
            for ti, (ps, pd, T, slot_base) in enumerate(schedule):
                Tb = T // 128
                icol0 = slot_base // 16
                rcol0 = slot_base // 128
                zs = gpool.tile([P, (BIG_T // 128) * D], mybir.dt.float32, tag="zs")
                zd = gpool.tile([P, (BIG_T // 128) * D], mybir.dt.float32, tag="zd")
                for buf, par, itile in ((zs, ps, isrc_sb), (zd, pd, idst_sb)):
                    nc.gpsimd.dma_gather(
                        out_ap=buf[:, : Tb * D].rearrange("p (k d) -> p k d", d=D),
                        in_ap=zview[:, par * D:(par + 1) * D],
                        idxs_ap=itile[:, icol0:icol0 + T // 16],
                        num_idxs=T,
                        num_idxs_reg=T,
                        elem_size=D,
                        elem_step=2 * D,
                        queue_num=0,
                        single_packet=SINGLE_PACKET,
                    )
                if compute:
                    prod = gpool.tile([P, (BIG_T // 128) * D], mybir.dt.float32,
                                      tag="prod")
                    nc.vector.tensor_tensor(
                        out=prod[:, : Tb * D],
                        in0=zs[:, : Tb * D],
                        in1=zd[:, : Tb * D],
                        op=mybir.AluOpType.mult,
                    )
                    if ACT_BLOCKS and (Tb == BIG_T // 128):
                        nb = ACT_BLOCKS
                        nc.vector.reduce_sum(
                            out=res[:, rcol0:rcol0 + Tb - nb],
                            in_=prod[:, : (Tb - nb) * D].rearrange(
                                "p (k d) -> p k d", d=D),
                            axis=mybir.AxisListType.X,
                        )
                        for bi in range(nb):
                            blk = Tb - nb + bi
                            nc.scalar.activation(
                                out=act_scratch[:, : D],
                                in_=prod[:, blk * D:(blk + 1) * D],
                                func=mybir.ActivationFunctionType.Copy,
                                accum_out=res[:, rcol0 + blk:rcol0 + blk + 1],
                            )
                    else:
                        nc.vector.reduce_sum(
                            out=res[:, rcol0:rcol0 + Tb],
                            in_=prod[:, : Tb * D].rearrange("p (k d) -> p k d", d=D),
                            axis=mybir.AxisListType.X,
                        )

            yt = mpool.tile([P, rcols], mybir.dt.float32, tag="yt")
            nc.scalar.activation(
                out=yt[:],
                in_=res[:],
                func=mybir.ActivationFunctionType.Sigmoid,
                bias=wb_dve[:, 1:2],
                scale=wb_dve[:, 0:1],
            )
            nc.sync.dma_start(out=Y[:], in_=yt[:])
    return nc


def _spread_gather_queues(nc):
    """Post-schedule: spread dma_gather descriptor generation over the 4 Q7
    core pairs. Tile assigns each SWDGE DMA a DMASW{L} completion-sem lane in
    scheduled order; a lane must be fed by a single queue, so queue = L %
    NUM_QUEUES keeps the lane->queue map consistent while rotating work
    across queues."""
    if NUM_QUEUES == 1:
        return
    for inst in nc.inst_map.values():
        if not isinstance(inst, mybir.InstDMAGatherAnt):
            continue
        si = inst.sync_info
        if si is None or not si.on_update:
            continue
        name = si.on_update[0].ant_name or ""
        if name.startswith("DMASW"):
            lane = int(name[5:].split("_")[0])
            fn = QUEUE_FROM_LANE or (lambda L: L % NUM_QUEUES)
            inst.queue_num = fn(lane)


def _prepare(Z, edge_index, w, b):
    """Host-side sharding/packing. Returns (in_maps, s2e_list, tiles, slots, E, Nn)."""
    Z = np.ascontiguousarray(np.asarray(Z, dtype=np.float32))
    ei = np.asarray(edge_index)
    w = np.asarray(w, dtype=np.float32).reshape(-1)[0]
    b = np.asarray(b, dtype=np.float32).reshape(-1)[0]
    n_nodes = Z.shape[0]
    E = ei.shape[1]
    src_all = ei[0].astype(np.int32)
    dst_all = ei[1].astype(np.int32)
    per_core = (E + N_CORES - 1) // N_CORES

    cores = []
    seg_counts = np.zeros((N_CORES, 4), np.int64)
    for c in range(N_CORES):
        lo = c * per_core
        hi = min(E, lo + per_core)
        s = src_all[lo:hi]
        d = dst_all[lo:hi]
        g = ((s & 1) << 1) | (d & 1)
        perm = np.argsort(g, kind="stable")
        cores.append((lo, s, d, g, perm))
        seg_counts[c] = np.bincount(g, minlength=4)

    caps = seg_counts.max(axis=0)
    seg_tiles = [_plan_tiles(int(caps[gg])) for gg in range(4)]
    seg_cap = [sum(ts) for ts in seg_tiles]
    slots = int(sum(seg_cap))
    tiles = []
    for gg in range(4):
        tiles += [(gg >> 1, gg & 1, T) for T in seg_tiles[gg]]

    wb = np.stack([np.full(P, w), np.full(P, b)], axis=1).astype(np.float32)
    in_maps = []
    s2e_list = []
    for c in range(N_CORES):
        lo, s, d, g, perm = cores[c]
        qs = np.zeros(slots, np.int16)
        qd = np.zeros(slots, np.int16)
        s2e = np.full(slots, -1, np.int64)
        off = 0
        gp = g[perm]
        for gg in range(4):
            e = perm[gp == gg]
            n = len(e)
            qs[off:off + n] = (s[e] >> 1).astype(np.int16)
            qd[off:off + n] = (d[e] >> 1).astype(np.int16)
            s2e[off:off + n] = lo + e
            off += seg_cap[gg]
        in_maps.append({
            "Z": Z,
            "isrc": np.ascontiguousarray(qs.reshape(-1, 16).T),
            "idst": np.ascontiguousarray(qd.reshape(-1, 16).T),
            "wb": wb,
        })
        s2e_list.append(s2e)
    return in_maps, s2e_list, tiles, slots, E, n_nodes


def _postprocess(results, s2e_list, E):
    Y = np.empty(E, np.float32)
    for c in range(N_CORES):
        yslots = results[c]["y"].T.ravel()
        s2e = s2e_list[c]
        valid = s2e >= 0
        Y[s2e[valid]] = yslots[valid]
    return Y[:, None]


def kernel(Z, edge_index, w, b, _trace=False, _trace_kwargs=None):
    in_maps, s2e_list, tiles, slots, E, n_nodes = _prepare(Z, edge_index, w, b)
    nc = bacc.Bacc("TRN2", num_swdge_queues=NUM_QUEUES,
                   dynamic_dma_scratch_size=DMA_SCRATCH)
    _build(nc, n_nodes, tiles, slots)
    _spread_gather_queues(nc)
    nc.finalize()
    run = run_bass_kernel_spmd(
        nc,
        in_maps,
        core_ids=list(range(N_CORES)),
        trace=_trace,
        **(_trace_kwargs or {}),
    )
    out = _postprocess(run.results, s2e_list, E)
    if _trace:
        return out, run
    return out
